# revision 25
# baseline (speedup 1.0000x reference)
"""AAFM sparse-attention kernel for 8 TRN2 NeuronCores.

Math (per batch b):
    qp = q @ Wq.T + bq ; kp = k @ Wk.T (+bk) ; vp = v @ Wv.T (+bv)
    q_sig = sigmoid(qp)
    exp_a = exp(-alpha * log2(Sk) * distances)        # [Sq, Sk]
    exp_k = exp(kp)                                   # [Sk, D]
    out   = q_sig * (exp_a @ (exp_k * vp)) / (exp_a @ exp_k)

Two algebraic simplifications (exact in real arithmetic):
  - bk cancels: exp(kp+bk) = exp(kp)*exp(bk), and exp(bk)[e] factors out of
    both numerator and denominator -> never computed.
  - bv pulls out: num = A@(ek*vp') + bv*den  =>  att = num'/den + bv, applied
    as a cheap epilogue add instead of a per-k-row bias.

Sharding: data-parallel over batch B=8, one batch per core; no collectives.
Host-side work is layout only (transposes, bias row replication) plus folding
alpha into one scalar.

Per-core structure (all matmuls bf16 via on-chip DVE casts, fp32 PSUM):
  Phase A (16 s-tiles): 12 projection MMs (K=128,N=512) per tile; ScalarE
    exp(kp) straight from PSUM -> resident B=[ek*vp'|ek] bf16 [128,16,1024];
    tanh((qp+bq)/2) -> TQ (sigmoid via tanh keeps ACT on the exp table set).
  Phase B (16 q-tiles): ScalarE exp(scale*dT) -> bf16, 32 accumulating MMs
    [q,128k]@[128k,2*512] -> num|den, DVE epilogue
    (tanh+1) * (num*0.5*recip_approx(den) + bv/2), batched out DMA.
DMA: Sync HWDGE ring carries qkv (1MB f32 group loads) + distances (1MB);
Scalar HWDGE ring carries weights (256KB chunks) + biases + outputs. A dummy
MM chain on memset tiles warms the PE clock during the startup DMA wait.
"""

import math
import sys

import numpy as np

sys.path.insert(0, "/opt/trn_rl_repo")

import concourse.bass as bass  # noqa: E402
import concourse.tile as tile  # noqa: E402
from concourse import bacc, mybir  # noqa: E402
from concourse.bass_utils import run_bass_kernel_spmd  # noqa: E402

P = 128
D = 512
S = 2048
B = 8
N_CORES = 8
DC = D // P  # 4 contraction chunks for projections

F32 = mybir.dt.float32
BF16 = mybir.dt.bfloat16
AF = mybir.ActivationFunctionType
ALU = mybir.AluOpType


def build_graph(exp_scale: float, s: int = S):
    """Build the single-core Bass/Tile graph. Same graph runs SPMD on 8 cores."""
    nt = s // P  # s-tiles == k-chunks == q-tiles
    ga = min(4, nt)  # s-tiles per qkv group DMA
    nc = bacc.Bacc(
        "TRN2",
        target_bir_lowering=False,
        debug=False,
        enable_asserts=True,
        num_devices=N_CORES,
    )

    qT = nc.dram_tensor("qT", [D, s], F32, kind="ExternalInput").ap()
    kT = nc.dram_tensor("kT", [D, s], F32, kind="ExternalInput").ap()
    vT = nc.dram_tensor("vT", [D, s], F32, kind="ExternalInput").ap()
    dT = nc.dram_tensor("dT", [s, s], F32, kind="ExternalInput").ap()
    wq = nc.dram_tensor("wq", [D, D], F32, kind="ExternalInput").ap()
    wk = nc.dram_tensor("wk", [D, D], F32, kind="ExternalInput").ap()
    wv = nc.dram_tensor("wv", [D, D], F32, kind="ExternalInput").ap()
    bq = nc.dram_tensor("bq", [P, D], F32, kind="ExternalInput").ap()
    bv = nc.dram_tensor("bv", [P, D], F32, kind="ExternalInput").ap()
    out = nc.dram_tensor("out", [s, D], F32, kind="ExternalOutput").ap()

    qT_r = qT.rearrange("(c p) s -> p c s", p=P)
    kT_r = kT.rearrange("(c p) s -> p c s", p=P)
    vT_r = vT.rearrange("(c p) s -> p c s", p=P)
    dT_r = dT.rearrange("(c p) q -> p c q", p=P)
    out_r = out.rearrange("(t p) e -> p t e", p=P)

    def mm(ps_ap, lhsT, rhs, start, stop):
        nc.tensor.matmul(ps_ap, lhsT, rhs, start=start, stop=stop)

    with tile.TileContext(nc) as tc:
        with (
            tc.tile_pool(name="consts", bufs=1) as consts,
            tc.tile_pool(name="wstage", bufs=3) as wstage,
            tc.tile_pool(name="resident", bufs=1) as resident,
            tc.tile_pool(name="stageA", bufs=2) as stageA,
            tc.tile_pool(name="stageB", bufs=2) as stageB,
            tc.tile_pool(name="tmpA", bufs=3) as tmpA,
            tc.tile_pool(name="tmpB", bufs=2) as tmpB,
            tc.tile_pool(name="outp", bufs=2) as outp,
            tc.tile_pool(name="psA", bufs=4, space="PSUM") as psA,
            tc.tile_pool(name="psB", bufs=2, space="PSUM") as psB,
        ):
            # Warm the ACT exp table set + PE clock during startup DMA wait.
            warm = consts.tile([P, D], BF16, tag="warm")
            nc.vector.memset(warm[:], 0.001)
            wexp = consts.tile([P, 1], F32, tag="wexp")
            nc.vector.memset(wexp[:], 0.0)
            nc.scalar.activation(wexp[:], wexp[:], AF.Exp)
            # Sized to bridge the HBM-bound phase-A fill deficit (~8.5us):
            # keeps the PE HAM clock-gate warm so real MMs run at 2.4 GHz.
            NDUMMY = 28
            wps = psA.tile([P, D], F32, tag="ps")
            for w in range(NDUMMY):
                mm(wps[:], warm[:, 0:P], warm[:], w == 0, w == NDUMMY - 1)

            # Weights: 256KB f32 chunks on Scalar ring, DVE cast to bf16.
            w_sb = {}
            for name, drm in (("wk", wk), ("wv", wv), ("wq", wq)):
                t = consts.tile([P, DC, D], BF16, tag=f"w_{name}")
                drm_r = drm.rearrange("(c p) e -> p c e", p=P)
                for c in range(DC):
                    st = wstage.tile([P, D], F32, tag="wstage")
                    nc.scalar.dma_start(st[:], drm_r[:, c, :])
                    nc.vector.tensor_copy(t[:, c, :], st[:])
                w_sb[name] = t
            bq_sb = consts.tile([P, D], F32, tag="bq")
            nc.scalar.dma_start(bq_sb[:], bq[:])
            bvh = consts.tile([P, D], F32, tag="bvh")
            nc.scalar.dma_start(bvh[:], bv[:])
            nc.vector.tensor_scalar_mul(bvh[:], bvh[:], 0.5)

            # B = [ek*vp' | ek], k on partitions, chunk i holds rows
            # k = i*128+p. TQ = tanh((qp+bq)/2) per q-tile.
            Bm = resident.tile([P, nt, 2 * D], BF16)
            TQ = resident.tile([P, nt, D], BF16)

            # ---- Phase A: projections, exp_k, B build ----
            # Projection waves: each tensor's group DMA unlocks a full wave of
            # MMs immediately (phase A is HBM-window-bound; don't gate PE on
            # all three tensors arriving).
            for g in range(nt // ga):
                gsl = bass.ts(g, ga * P)
                qkv_f32 = {}
                for nm, src in (("k", kT_r), ("v", vT_r), ("q", qT_r)):
                    t = stageA.tile([P, DC, ga * P], F32, tag=f"{nm}f32")
                    nc.sync.dma_start(t[:], src[:, :, gsl])
                    qkv_f32[nm] = t

                for nm, wname in (("k", "wk"), ("v", "wv"), ("q", "wq")):
                    pss = []
                    for ii in range(ga):
                        # DVE cast to packed per-s-tile bf16 (256B LDW stride)
                        a = stageA.tile([P, DC, P], BF16, tag=f"{nm}a")
                        nc.vector.tensor_copy(
                            a[:], qkv_f32[nm][:, :, bass.ts(ii, P)]
                        )
                        p = psA.tile([P, D], F32, tag="ps")
                        pss.append(p)
                        for c in range(DC):
                            mm(p[:], a[:, c, :], w_sb[wname][:, c, :], c == 0, c == DC - 1)
                    for ii in range(ga):
                        i = g * ga + ii
                        if nm == "k":
                            # ek = exp(kp) from PSUM -> B[:, i, D:2D]
                            nc.scalar.activation(
                                Bm[:, i, D : 2 * D], pss[ii][:], AF.Exp
                            )
                        elif nm == "v":
                            # ek * vp' -> B[:, i, 0:D]
                            nc.vector.tensor_mul(
                                Bm[:, i, 0:D], Bm[:, i, D : 2 * D], pss[ii][:]
                            )
                        else:
                            # tanh((qp + bq)/2) -> TQ[:, i, :]
                            qpb = tmpA.tile([P, D], F32, tag="qpb")
                            nc.vector.tensor_add(qpb[:], pss[ii][:], bq_sb[:])
                            nc.scalar.activation(
                                TQ[:, i, :], qpb[:], AF.Tanh, scale=0.5
                            )

            # ---- Phase B: exp_a, attention matmul, epilogue ----
            for j in range(nt):
                da = stageB.tile([P, nt, P], F32, tag="da")
                nc.sync.dma_start(da[:], dT_r[:, :, bass.ts(j, P)])
                ea = stageB.tile([P, nt, P], BF16, tag="ea")
                nc.scalar.activation(ea[:], da[:], AF.Exp, scale=exp_scale)

                ps = psB.tile([P, 2, D], F32, tag="att")
                r = tmpB.tile([P, D], F32, tag="recip")
                rq = tmpB.tile([P, D], F32, tag="rq")
                tqb = tmpB.tile([P, D], F32, tag="tqb")
                # den group (hh=1) first: recip + epilogue prep overlap num MMs
                for hh in (1, 0):
                    for c in range(nt):
                        mm(
                            ps[:, hh, :],
                            ea[:, c, :],
                            Bm[:, c, bass.ts(hh, D)],
                            c == 0,
                            c == nt - 1,
                        )
                    if hh == 1:
                        nc.vector.reciprocal_approx_fast(r[:], ps[:, 1, :])
                        # rq = (tanh+1) * 0.5/den ; tqb = (tanh+1) * bv/2
                        nc.vector.scalar_tensor_tensor(
                            rq[:], TQ[:, j, :], 1.0, r[:], op0=ALU.add, op1=ALU.mult
                        )
                        nc.vector.tensor_scalar_mul(rq[:], rq[:], 0.5)
                        nc.vector.scalar_tensor_tensor(
                            tqb[:], TQ[:, j, :], 1.0, bvh[:], op0=ALU.add, op1=ALU.mult
                        )
                # out = num*rq + tqb  ==  sigmoid(qp) * (num/den + bv)
                na = tmpB.tile([P, D], F32, tag="na")
                nc.vector.tensor_mul(na[:], ps[:, 0, :], rq[:])
                ot = outp.tile([P, D], F32, tag="ot")
                nc.vector.tensor_add(ot[:], na[:], tqb[:])
                nc.scalar.dma_start(out_r[:, j, :], ot[:])

    nc.compile()
    return nc


def make_in_maps(q, k, v, distances, Wq, bq, Wk, bk, Wv, bv):
    """Per-core input maps: layout-only host work (transposes, bias tiling)."""
    wq_t = np.ascontiguousarray(Wq.T)  # [d, e]
    wk_t = np.ascontiguousarray(Wk.T)
    wv_t = np.ascontiguousarray(Wv.T)
    bq_t = np.ascontiguousarray(np.broadcast_to(bq[None, :], (P, D)))
    bv_t = np.ascontiguousarray(np.broadcast_to(bv[None, :], (P, D)))
    in_maps = []
    for b in range(B):
        in_maps.append(
            {
                "qT": np.ascontiguousarray(q[b].T),
                "kT": np.ascontiguousarray(k[b].T),
                "vT": np.ascontiguousarray(v[b].T),
                "dT": np.ascontiguousarray(distances[b].T),
                "wq": wq_t,
                "wk": wk_t,
                "wv": wv_t,
                "bq": bq_t,
                "bv": bv_t,
            }
        )
    return in_maps


def _exp_scale(alpha, n):
    # mirror reference: log2_n = log(n)/log(2) in fp32, bias = -alpha*log2_n*d
    log2_n = np.float32(np.log(np.float32(n))) / np.float32(np.log(np.float32(2.0)))
    return float(np.float32(-np.float32(alpha) * log2_n))


_GRAPH_CACHE = {}


def run(q, k, v, distances, Wq, bq, Wk, bk, Wv, bv, alpha, trace=False, tmpdir=None):
    scale = _exp_scale(alpha[0], k.shape[1])
    key = scale
    if key not in _GRAPH_CACHE:
        _GRAPH_CACHE[key] = build_graph(scale)
    nc = _GRAPH_CACHE[key]
    in_maps = make_in_maps(q, k, v, distances, Wq, bq, Wk, bk, Wv, bv)
    res = run_bass_kernel_spmd(
        nc, in_maps, core_ids=list(range(N_CORES)), trace=trace, tmpdir=tmpdir
    )
    outs = np.stack([res.results[b]["out"] for b in range(B)], axis=0)
    return outs.astype(np.float32), res


def kernel(q, k, v, distances, Wq, bq, Wk, bk, Wv, bv, alpha):
    out, _ = run(q, k, v, distances, Wq, bq, Wk, bk, Wv, bv, alpha, trace=False)
    return out


# revision 28
# speedup vs baseline: 1.1923x; 1.1923x over previous
"""AAFM sparse-attention kernel for 8 TRN2 NeuronCores.

Math (per batch b):
    qp = q @ Wq.T + bq ; kp = k @ Wk.T (+bk) ; vp = v @ Wv.T (+bv)
    q_sig = sigmoid(qp)
    exp_a = exp(-alpha * log2(Sk) * distances)        # [Sq, Sk]
    exp_k = exp(kp)                                   # [Sk, D]
    out   = q_sig * (exp_a @ (exp_k * vp)) / (exp_a @ exp_k)

Two algebraic simplifications (exact in real arithmetic):
  - bk cancels: exp(kp+bk) = exp(kp)*exp(bk), and exp(bk)[e] factors out of
    both numerator and denominator -> never computed.
  - bv pulls out: num = A@(ek*vp') + bv*den  =>  att = num'/den + bv, applied
    as a cheap epilogue add instead of a per-k-row bias.

Sharding: data-parallel over batch B=8, one batch per core; no collectives.
Host-side work is layout only (transposes, bias row replication) plus folding
alpha into one scalar.

Per-core structure (all matmuls bf16 via on-chip DVE casts, fp32 PSUM):
  Phase A (16 s-tiles): 12 projection MMs (K=128,N=512) per tile; ScalarE
    exp(kp) straight from PSUM -> resident B=[ek*vp'|ek] bf16 [128,16,1024];
    tanh((qp+bq)/2) -> TQ (sigmoid via tanh keeps ACT on the exp table set).
  Phase B (16 q-tiles): ScalarE exp(scale*dT) -> bf16, 32 accumulating MMs
    [q,128k]@[128k,2*512] -> num|den, DVE epilogue
    (tanh+1) * (num*0.5*recip_approx(den) + bv/2), batched out DMA.
DMA: Sync HWDGE ring carries qkv (1MB f32 group loads) + distances (1MB);
Scalar HWDGE ring carries weights (256KB chunks) + biases + outputs. A dummy
MM chain on memset tiles warms the PE clock during the startup DMA wait.
"""

import math
import sys

import numpy as np

sys.path.insert(0, "/opt/trn_rl_repo")

import concourse.bass as bass  # noqa: E402
import concourse.tile as tile  # noqa: E402
from concourse import bacc, mybir  # noqa: E402
from concourse.bass_utils import run_bass_kernel_spmd  # noqa: E402

P = 128
D = 512
S = 2048
B = 8
N_CORES = 8
DC = D // P  # 4 contraction chunks for projections

F32 = mybir.dt.float32
BF16 = mybir.dt.bfloat16
AF = mybir.ActivationFunctionType
ALU = mybir.AluOpType


def build_graph(exp_scale: float, s: int = S):
    """Build the single-core Bass/Tile graph. Same graph runs SPMD on 8 cores."""
    nt = s // P  # s-tiles == k-chunks == q-tiles
    ga = min(2, nt)  # s-tiles per qkv group DMA (512KB: finer fill arrivals)
    nc = bacc.Bacc(
        "TRN2",
        target_bir_lowering=False,
        debug=False,
        enable_asserts=True,
        num_devices=N_CORES,
    )

    qT = nc.dram_tensor("qT", [D, s], F32, kind="ExternalInput").ap()
    kT = nc.dram_tensor("kT", [D, s], F32, kind="ExternalInput").ap()
    vT = nc.dram_tensor("vT", [D, s], F32, kind="ExternalInput").ap()
    dT = nc.dram_tensor("dT", [s, s], F32, kind="ExternalInput").ap()
    wq = nc.dram_tensor("wq", [D, D], F32, kind="ExternalInput").ap()
    wk = nc.dram_tensor("wk", [D, D], F32, kind="ExternalInput").ap()
    wv = nc.dram_tensor("wv", [D, D], F32, kind="ExternalInput").ap()
    bq = nc.dram_tensor("bq", [P, D], F32, kind="ExternalInput").ap()
    bv = nc.dram_tensor("bv", [P, D], F32, kind="ExternalInput").ap()
    out = nc.dram_tensor("out", [s, D], F32, kind="ExternalOutput").ap()

    qT_r = qT.rearrange("(c p) s -> p c s", p=P)
    kT_r = kT.rearrange("(c p) s -> p c s", p=P)
    vT_r = vT.rearrange("(c p) s -> p c s", p=P)
    dT_r = dT.rearrange("(c p) q -> p c q", p=P)
    out_r = out.rearrange("(t p) e -> p t e", p=P)

    def mm(ps_ap, lhsT, rhs, start, stop):
        nc.tensor.matmul(ps_ap, lhsT, rhs, start=start, stop=stop)

    with tile.TileContext(nc) as tc:
        with (
            tc.tile_pool(name="consts", bufs=1) as consts,
            tc.tile_pool(name="wstage", bufs=3) as wstage,
            tc.tile_pool(name="resident", bufs=1) as resident,
            tc.tile_pool(name="stageA", bufs=2) as stageA,
            tc.tile_pool(name="stageB", bufs=2) as stageB,
            tc.tile_pool(name="tmpA", bufs=3) as tmpA,
            tc.tile_pool(name="tmpB", bufs=2) as tmpB,
            tc.tile_pool(name="outp", bufs=2) as outp,
            tc.tile_pool(name="psA", bufs=4, space="PSUM") as psA,
            tc.tile_pool(name="psB", bufs=2, space="PSUM") as psB,
        ):
            # Warm the ACT exp table set + PE clock during startup DMA wait.
            warm = consts.tile([P, D], BF16, tag="warm")
            nc.vector.memset(warm[:], 0.001)
            wexp = consts.tile([P, 1], F32, tag="wexp")
            nc.vector.memset(wexp[:], 0.0)
            nc.scalar.activation(wexp[:], wexp[:], AF.Exp)
            # Sized to bridge the HBM-bound phase-A fill deficit (~8.5us):
            # keeps the PE HAM clock-gate warm so real MMs run at 2.4 GHz.
            NDUMMY = 28
            wps = psA.tile([P, D], F32, tag="ps")
            for w in range(NDUMMY):
                mm(wps[:], warm[:, 0:P], warm[:], w == 0, w == NDUMMY - 1)

            # Weights: 256KB f32 chunks on Scalar ring, DVE cast to bf16.
            w_sb = {}
            for name, drm in (("wk", wk), ("wv", wv), ("wq", wq)):
                t = consts.tile([P, DC, D], BF16, tag=f"w_{name}")
                drm_r = drm.rearrange("(c p) e -> p c e", p=P)
                for c in range(DC):
                    st = wstage.tile([P, D], F32, tag="wstage")
                    nc.scalar.dma_start(st[:], drm_r[:, c, :])
                    nc.vector.tensor_copy(t[:, c, :], st[:])
                w_sb[name] = t
            bq_sb = consts.tile([P, D], F32, tag="bq")
            nc.scalar.dma_start(bq_sb[:], bq[:])
            bvh = consts.tile([P, D], F32, tag="bvh")
            nc.scalar.dma_start(bvh[:], bv[:])
            nc.vector.tensor_scalar_mul(bvh[:], bvh[:], 0.5)

            # B = [ek*vp' | ek], k on partitions, chunk i holds rows
            # k = i*128+p. TQ = tanh((qp+bq)/2) per q-tile.
            Bm = resident.tile([P, nt, 2 * D], BF16)
            TQ = resident.tile([P, nt, D], BF16)

            # ---- Phase A: projections, exp_k, B build ----
            # Projection waves: each tensor's group DMA unlocks a full wave of
            # MMs immediately (phase A is HBM-window-bound; don't gate PE on
            # all three tensors arriving).
            for g in range(nt // ga):
                gsl = bass.ts(g, ga * P)
                qkv_f32 = {}
                for nm, src in (("k", kT_r), ("v", vT_r), ("q", qT_r)):
                    t = stageA.tile([P, DC, ga * P], F32, tag=f"{nm}f32")
                    nc.sync.dma_start(t[:], src[:, :, gsl])
                    qkv_f32[nm] = t

                for nm, wname in (("k", "wk"), ("v", "wv"), ("q", "wq")):
                    pss = []
                    for ii in range(ga):
                        # DVE cast to packed per-s-tile bf16 (256B LDW stride)
                        a = stageA.tile([P, DC, P], BF16, tag=f"{nm}a")
                        nc.vector.tensor_copy(
                            a[:], qkv_f32[nm][:, :, bass.ts(ii, P)]
                        )
                        p = psA.tile([P, D], F32, tag="ps")
                        pss.append(p)
                        for c in range(DC):
                            mm(p[:], a[:, c, :], w_sb[wname][:, c, :], c == 0, c == DC - 1)
                    for ii in range(ga):
                        i = g * ga + ii
                        if nm == "k":
                            # ek = exp(kp) from PSUM -> B[:, i, D:2D]
                            nc.scalar.activation(
                                Bm[:, i, D : 2 * D], pss[ii][:], AF.Exp
                            )
                        elif nm == "v":
                            # 0.5*ek*vp' -> B[:, i, 0:D] (0.5 pre-folds the
                            # sigmoid half so the epilogue saves an op; it
                            # cancels nowhere else since num/den keeps den raw)
                            nc.vector.scalar_tensor_tensor(
                                Bm[:, i, 0:D],
                                Bm[:, i, D : 2 * D],
                                0.5,
                                pss[ii][:],
                                op0=ALU.mult,
                                op1=ALU.mult,
                            )
                        else:
                            # tanh((qp + bq)/2) -> TQ[:, i, :]
                            qpb = tmpA.tile([P, D], F32, tag="qpb")
                            nc.vector.tensor_add(qpb[:], pss[ii][:], bq_sb[:])
                            nc.scalar.activation(
                                TQ[:, i, :], qpb[:], AF.Tanh, scale=0.5
                            )

            # ---- Phase B: exp_a, attention matmul, epilogue ----
            for j in range(nt):
                da = stageB.tile([P, nt, P], F32, tag="da")
                nc.sync.dma_start(da[:], dT_r[:, :, bass.ts(j, P)])
                ea = stageB.tile([P, nt, P], BF16, tag="ea")
                nc.scalar.activation(ea[:], da[:], AF.Exp, scale=exp_scale)

                ps = psB.tile([P, 2, D], F32, tag="att")
                r = tmpB.tile([P, D], F32, tag="recip")
                rq = tmpB.tile([P, D], F32, tag="rq")
                tqb = tmpB.tile([P, D], F32, tag="tqb")
                # den group (hh=1) first: recip + epilogue prep overlap num MMs
                for hh in (1, 0):
                    for c in range(nt):
                        mm(
                            ps[:, hh, :],
                            ea[:, c, :],
                            Bm[:, c, bass.ts(hh, D)],
                            c == 0,
                            c == nt - 1,
                        )
                    if hh == 1:
                        nc.vector.reciprocal_approx_fast(r[:], ps[:, 1, :])
                        # rq = (tanh+1)/den ; tqb = (tanh+1) * bv/2
                        # (num PSUM already carries the 0.5 from phase A)
                        nc.vector.scalar_tensor_tensor(
                            rq[:], TQ[:, j, :], 1.0, r[:], op0=ALU.add, op1=ALU.mult
                        )
                        nc.vector.scalar_tensor_tensor(
                            tqb[:], TQ[:, j, :], 1.0, bvh[:], op0=ALU.add, op1=ALU.mult
                        )
                # out = num*rq + tqb  ==  sigmoid(qp) * (num/den + bv)
                na = tmpB.tile([P, D], F32, tag="na")
                nc.vector.tensor_mul(na[:], ps[:, 0, :], rq[:])
                ot = outp.tile([P, D], F32, tag="ot")
                nc.vector.tensor_add(ot[:], na[:], tqb[:])
                nc.scalar.dma_start(out_r[:, j, :], ot[:])

    nc.compile()
    return nc


def make_in_maps(q, k, v, distances, Wq, bq, Wk, bk, Wv, bv):
    """Per-core input maps: layout-only host work (transposes, bias tiling)."""
    wq_t = np.ascontiguousarray(Wq.T)  # [d, e]
    wk_t = np.ascontiguousarray(Wk.T)
    wv_t = np.ascontiguousarray(Wv.T)
    bq_t = np.ascontiguousarray(np.broadcast_to(bq[None, :], (P, D)))
    bv_t = np.ascontiguousarray(np.broadcast_to(bv[None, :], (P, D)))
    in_maps = []
    for b in range(B):
        in_maps.append(
            {
                "qT": np.ascontiguousarray(q[b].T),
                "kT": np.ascontiguousarray(k[b].T),
                "vT": np.ascontiguousarray(v[b].T),
                "dT": np.ascontiguousarray(distances[b].T),
                "wq": wq_t,
                "wk": wk_t,
                "wv": wv_t,
                "bq": bq_t,
                "bv": bv_t,
            }
        )
    return in_maps


def _exp_scale(alpha, n):
    # mirror reference: log2_n = log(n)/log(2) in fp32, bias = -alpha*log2_n*d
    log2_n = np.float32(np.log(np.float32(n))) / np.float32(np.log(np.float32(2.0)))
    return float(np.float32(-np.float32(alpha) * log2_n))


_GRAPH_CACHE = {}


def run(q, k, v, distances, Wq, bq, Wk, bk, Wv, bv, alpha, trace=False, tmpdir=None):
    scale = _exp_scale(alpha[0], k.shape[1])
    key = scale
    if key not in _GRAPH_CACHE:
        _GRAPH_CACHE[key] = build_graph(scale)
    nc = _GRAPH_CACHE[key]
    in_maps = make_in_maps(q, k, v, distances, Wq, bq, Wk, bk, Wv, bv)
    res = run_bass_kernel_spmd(
        nc, in_maps, core_ids=list(range(N_CORES)), trace=trace, tmpdir=tmpdir
    )
    outs = np.stack([res.results[b]["out"] for b in range(B)], axis=0)
    return outs.astype(np.float32), res


def kernel(q, k, v, distances, Wq, bq, Wk, bk, Wv, bv, alpha):
    out, _ = run(q, k, v, distances, Wq, bq, Wk, bk, Wv, bv, alpha, trace=False)
    return out
